# revision 2
# baseline (speedup 1.0000x reference)
"""Grouped-GEMM (MoE routing) kernel for TRN2, 8 NeuronCores, SPMD.

out[m] = values[m] @ combining_matrix[species_idx[m]]
  values [131072, 128] f32, species_idx [131072] i32, combining_matrix [8, 128, 256] f32

Strategy:
  - Host: counting-sort rows by species; deal each species' rows round-robin
    across the 8 cores so per-core per-species counts are balanced (+-1).
    Each core's rows are packed species-contiguous into a transposed buffer
    xT [128, R_pad] (species segment s zero-padded to a static capacity C[s],
    identical on every core -> one SPMD program).
  - fp16 end-to-end I/O: x, weights and the output cross HBM as fp16 (the
    kernel is DMA-bound; fp32 accumulate in PSUM keeps rel-err ~5e-4).
  - Device (per core): x lives in one persistent SBUF tile [128, R_pad]
    streamed in by a few 4096-col DMAs (8KB/partition lines). All 8 weight
    matrices resident in SBUF ([128, 8*256] fp16 = 4KB/partition). For each
    species s, output half h and 2048-col supertile: 4 matmuls (512-col
    moving chunks, K=128 contraction) into a 4-bank PSUM tile, one
    fp32->fp16 copy to SBUF, one DMA to outT [256, R_pad].
  - Host: scatter outT columns back to the full [131072, 256] fp32 output.

This does 1x the FLOPs of the reference's 8x masked-matmul formulation and is
DMA-roofline-bound (~13.3 MB/core HBM traffic at fp16).
"""

import numpy as np
from contextlib import ExitStack

import concourse.bass as bass
import concourse.mybir as mybir
import concourse.tile as tile
from concourse import bacc
from concourse.bass_utils import run_bass_kernel_spmd

M_TOTAL = 131072
D_IN = 128
N_OUT = 256
N_SPECIES = 8
N_CORES = 8
PAD = 64           # species segment capacity granularity (rows)
CHUNK = 512        # matmul moving-dim chunk (PSUM bank = 512 fp32 cols)
ST = 2048          # supertile: 4 PSUM banks -> one copy + one output DMA
IN_PIECE = 4096    # input DMA piece (cols); 8KB/partition lines at fp16
F32 = mybir.dt.float32
F16 = mybir.dt.float16


def _build_nc(caps, r_pad):
    """Build the SPMD program for one core. caps[s] = padded column count of
    species segment s (same on all cores); r_pad = sum(caps)."""
    nc = bacc.Bacc("TRN2", target_bir_lowering=False, debug=False,
                   num_devices=N_CORES)
    xT = nc.dram_tensor("xT", [D_IN, r_pad], F16, kind="ExternalInput").ap()
    w = nc.dram_tensor("w", [D_IN, N_SPECIES * N_OUT], F16,
                       kind="ExternalInput").ap()
    outT = nc.dram_tensor("outT", [N_OUT, r_pad], F16, kind="ExternalOutput").ap()

    # (species, xT column offset, columns) per species segment
    sched = []
    off = 0
    for s in range(N_SPECIES):
        if caps[s]:
            sched.append((s, off, caps[s]))
            off += caps[s]

    with tile.TileContext(nc) as tc, ExitStack() as ctx:
        wpool = ctx.enter_context(tc.tile_pool(name="w", bufs=1))
        xpool = ctx.enter_context(tc.tile_pool(name="x", bufs=1))
        opool = ctx.enter_context(tc.tile_pool(name="o", bufs=6))
        pspool = ctx.enter_context(tc.tile_pool(name="ps", bufs=2, space="PSUM"))

        wt = wpool.tile([D_IN, N_SPECIES * N_OUT], F16)
        xt = xpool.tile([D_IN, r_pad], F16)

        # weights first on the sync ring, first x piece on the (otherwise
        # idle) scalar ring so both HWDGE rings ramp in parallel at start
        nc.sync.dma_start(wt, w)
        pieces = [(0, min(ST, r_pad))]
        p0 = pieces[0][1]
        while p0 < r_pad:
            pn = min(IN_PIECE, r_pad - p0)
            pieces.append((p0, pn))
            p0 += pn
        for i, (p0, pn) in enumerate(pieces):
            ieng = nc.scalar if i == 0 else nc.sync
            ieng.dma_start(xt[:, p0:p0 + pn], xT[:, p0:p0 + pn])

        for (s, off, cs) in sched:
            for h in range(2):
                lhsT = wt[:, s * N_OUT + h * 128: s * N_OUT + h * 128 + 128]
                for t0 in range(0, cs, ST):
                    tn = min(ST, cs - t0)
                    ps = pspool.tile([128, ST], F32, tag="ps")
                    for j0 in range(0, tn, CHUNK):
                        cj = min(CHUNK, tn - j0)
                        nc.tensor.matmul(ps[:, j0:j0 + cj], lhsT,
                                         xt[:, off + t0 + j0:off + t0 + j0 + cj],
                                         start=True, stop=True)
                    ot = opool.tile([128, ST], F16, tag="o")
                    nc.vector.tensor_copy(ot[:, :tn], ps[:, :tn])
                    nc.scalar.dma_start(
                        outT[h * 128:(h + 1) * 128, off + t0:off + t0 + tn],
                        ot[:, :tn])

    nc.compile()
    return nc


def _prepare(values, species_idx, combining_matrix):
    """Host routing + packing. Returns (in_maps, plan)."""
    values = np.ascontiguousarray(values, dtype=np.float32)
    species_idx = np.asarray(species_idx, dtype=np.int32)
    w_host = np.ascontiguousarray(
        np.asarray(combining_matrix, dtype=np.float32).transpose(1, 0, 2).reshape(
            D_IN, N_SPECIES * N_OUT)
    ).astype(np.float16)

    # per species, deal rows round-robin across cores (balanced +-1)
    core_rows = [[] for _ in range(N_CORES)]   # per core: list of row-index arrays
    counts = np.zeros((N_CORES, N_SPECIES), dtype=np.int64)
    for s in range(N_SPECIES):
        idx = np.nonzero(species_idx == s)[0]
        for c in range(N_CORES):
            sub = idx[c::N_CORES]
            core_rows[c].append(sub)
            counts[c, s] = sub.size

    caps = []
    for s in range(N_SPECIES):
        mx = int(counts[:, s].max())
        caps.append(0 if mx == 0 else -(-mx // PAD) * PAD)
    r_pad = int(sum(caps))
    offs = np.concatenate([[0], np.cumsum(caps)]).astype(np.int64)

    in_maps = []
    for c in range(N_CORES):
        xT = np.zeros((D_IN, r_pad), dtype=np.float16)
        for s in range(N_SPECIES):
            n = counts[c, s]
            if n:
                xT[:, offs[s]:offs[s] + n] = values[core_rows[c][s]].T
        in_maps.append({"xT": xT, "w": w_host})

    plan = {"core_rows": core_rows, "counts": counts, "caps": caps,
            "offs": offs, "r_pad": r_pad}
    return in_maps, plan


def _postprocess(results, plan):
    core_rows, counts, offs = plan["core_rows"], plan["counts"], plan["offs"]
    out = np.empty((M_TOTAL, N_OUT), dtype=np.float32)
    for c in range(N_CORES):
        oT = results[c]["outT"].astype(np.float32)
        for s in range(N_SPECIES):
            n = counts[c, s]
            if n:
                out[core_rows[c][s]] = oT[:, offs[s]:offs[s] + n].T
    return out


def kernel(values, species_idx, combining_matrix):
    in_maps, plan = _prepare(values, species_idx, combining_matrix)
    nc = _build_nc(plan["caps"], plan["r_pad"])
    res = run_bass_kernel_spmd(nc, in_maps, list(range(N_CORES)))
    return _postprocess(res.results, plan)


# revision 3
# speedup vs baseline: 1.1797x; 1.1797x over previous
"""Grouped-GEMM (MoE routing) kernel for TRN2, 8 NeuronCores, SPMD.

out[m] = values[m] @ combining_matrix[species_idx[m]]
  values [131072, 128] f32, species_idx [131072] i32, combining_matrix [8, 128, 256] f32

Strategy:
  - Host: counting-sort rows by species; deal each species' rows round-robin
    across the 8 cores so per-core per-species counts are balanced (+-1).
    Each core's rows are packed species-contiguous into a transposed buffer
    xT [128, R_pad] (species segment s zero-padded to a static capacity C[s],
    identical on every core -> one SPMD program).
  - fp16 end-to-end I/O: x, weights and the output cross HBM as fp16 (the
    kernel is DMA-bound; fp32 accumulate in PSUM keeps rel-err ~5e-4).
  - Device (per core): x lives in one persistent SBUF tile [128, R_pad]
    streamed in by a few 4096-col DMAs (8KB/partition lines). All 8 weight
    matrices resident in SBUF ([128, 8*256] fp16 = 4KB/partition). Per
    species s / output half h, the segment is split into near-equal
    supertiles <=2048 cols; each runs 512-col matmuls into a 4-bank PSUM
    tile (bufs=2: uniform supertile sizes keep the two slots strictly
    alternating so PE and the cast engines overlap). PSUM fp32 -> fp16
    casts alternate between DVE and ACT, writing into 4096-col staging
    tiles; a completed staging tile is DMAed (8KB/partition lines) to
    outT [256, R_pad] from the otherwise-idle sync queue.
  - Host: scatter outT columns back to the full [131072, 256] fp32 output.

This does 1x the FLOPs of the reference's 8x masked-matmul formulation and is
DMA-roofline-bound (~13.3 MB/core HBM traffic at fp16).
"""

import numpy as np
from contextlib import ExitStack

import concourse.bass as bass
import concourse.mybir as mybir
import concourse.tile as tile
from concourse import bacc
from concourse.bass_utils import run_bass_kernel_spmd

M_TOTAL = 131072
D_IN = 128
N_OUT = 256
N_SPECIES = 8
N_CORES = 8
PAD = 64           # species segment capacity granularity (rows)
CHUNK = 512        # matmul moving-dim chunk (PSUM bank = 512 fp32 cols)
ST = 2048          # max supertile cols (4 PSUM banks)
IN_PIECE = 4096    # input DMA piece (cols); 8KB/partition lines at fp16
OPIECE = 4096      # output staging piece (cols); 8KB/partition lines
F32 = mybir.dt.float32
F16 = mybir.dt.float16


def _supertiles(cs):
    """Split cs cols into near-equal supertiles <= ST (returns sizes)."""
    n = -(-cs // ST)
    q, r = divmod(cs, n)
    return [q + (1 if i < r else 0) for i in range(n)]


def _build_nc(caps, r_pad):
    """Build the SPMD program for one core. caps[s] = padded column count of
    species segment s (same on all cores); r_pad = sum(caps)."""
    nc = bacc.Bacc("TRN2", target_bir_lowering=False, debug=False,
                   num_devices=N_CORES)
    xT = nc.dram_tensor("xT", [D_IN, r_pad], F16, kind="ExternalInput").ap()
    w = nc.dram_tensor("w", [D_IN, N_SPECIES * N_OUT], F16,
                       kind="ExternalInput").ap()
    outT = nc.dram_tensor("outT", [N_OUT, r_pad], F16, kind="ExternalOutput").ap()

    # (species, xT column offset, columns) per species segment
    sched = []
    off = 0
    for s in range(N_SPECIES):
        if caps[s]:
            sched.append((s, off, caps[s]))
            off += caps[s]

    # output staging pieces per h: first small so the output stream starts
    # early, then 8KB-line pieces
    opieces = [min(ST, r_pad)]
    while sum(opieces) < r_pad:
        opieces.append(min(OPIECE, r_pad - sum(opieces)))
    obounds = np.concatenate([[0], np.cumsum(opieces)]).astype(int)

    with tile.TileContext(nc) as tc, ExitStack() as ctx:
        wpool = ctx.enter_context(tc.tile_pool(name="w", bufs=1))
        xpool = ctx.enter_context(tc.tile_pool(name="x", bufs=1))
        opool = ctx.enter_context(tc.tile_pool(name="o", bufs=6))
        pspool = ctx.enter_context(tc.tile_pool(name="ps", bufs=2, space="PSUM"))

        wt = wpool.tile([D_IN, N_SPECIES * N_OUT], F16)
        xt = xpool.tile([D_IN, r_pad], F16)

        # weights first on the sync ring, first x piece on the (still idle)
        # scalar ring so both HWDGE rings ramp in parallel at kernel start
        nc.sync.dma_start(wt, w)
        pieces = [(0, min(ST, r_pad))]
        p0 = pieces[0][1]
        while p0 < r_pad:
            pn = min(IN_PIECE, r_pad - p0)
            pieces.append((p0, pn))
            p0 += pn
        for i, (p0, pn) in enumerate(pieces):
            ieng = nc.scalar if i == 0 else nc.sync
            ieng.dma_start(xt[:, p0:p0 + pn], xT[:, p0:p0 + pn])

        # per h: lazily-created staging tiles and #cols still to be cast
        otile = {}                      # (h, piece idx) -> SBUF tile
        left = {0: list(opieces), 1: list(opieces)}
        n_cast = 0

        def emit_cast(eng, h, g0, ps, ps0, tn):
            """Cast ps[:, ps0:ps0+tn] (fp32) into the h staging tiles at
            global col g0, splitting at piece boundaries; DMA filled pieces."""
            while tn:
                k = int(np.searchsorted(obounds, g0, side="right")) - 1
                c0 = g0 - obounds[k]
                n = min(tn, int(obounds[k + 1]) - g0)
                if (h, k) not in otile:
                    otile[(h, k)] = opool.tile([128, OPIECE], F16, tag="o",
                                               name=f"ot_{h}_{k}")
                ot = otile[(h, k)]
                eng_ = nc.vector if eng == 0 else nc.scalar
                if eng == 0:
                    eng_.tensor_copy(ot[:, c0:c0 + n], ps[:, ps0:ps0 + n])
                else:
                    eng_.copy(ot[:, c0:c0 + n], ps[:, ps0:ps0 + n])
                left[h][k] -= n
                if left[h][k] == 0:
                    pw = int(opieces[k])
                    nc.sync.dma_start(
                        outT[h * 128:(h + 1) * 128,
                             int(obounds[k]):int(obounds[k]) + pw],
                        ot[:, :pw])
                g0 += n
                ps0 += n
                tn -= n

        for (s, off, cs) in sched:
            for h in range(2):
                lhsT = wt[:, s * N_OUT + h * 128: s * N_OUT + h * 128 + 128]
                t0 = 0
                for tn in _supertiles(cs):
                    ps = pspool.tile([128, ST], F32, tag="ps")
                    for j0 in range(0, tn, CHUNK):
                        cj = min(CHUNK, tn - j0)
                        nc.tensor.matmul(ps[:, j0:j0 + cj], lhsT,
                                         xt[:, off + t0 + j0:off + t0 + j0 + cj],
                                         start=True, stop=True)
                    emit_cast(n_cast % 2, h, off + t0, ps, 0, tn)
                    n_cast += 1
                    t0 += tn

    nc.compile()
    return nc


def _prepare(values, species_idx, combining_matrix):
    """Host routing + packing. Returns (in_maps, plan)."""
    values = np.ascontiguousarray(values, dtype=np.float32)
    species_idx = np.asarray(species_idx, dtype=np.int32)
    w_host = np.ascontiguousarray(
        np.asarray(combining_matrix, dtype=np.float32).transpose(1, 0, 2).reshape(
            D_IN, N_SPECIES * N_OUT)
    ).astype(np.float16)

    # per species, deal rows round-robin across cores (balanced +-1)
    core_rows = [[] for _ in range(N_CORES)]   # per core: list of row-index arrays
    counts = np.zeros((N_CORES, N_SPECIES), dtype=np.int64)
    for s in range(N_SPECIES):
        idx = np.nonzero(species_idx == s)[0]
        for c in range(N_CORES):
            sub = idx[c::N_CORES]
            core_rows[c].append(sub)
            counts[c, s] = sub.size

    caps = []
    for s in range(N_SPECIES):
        mx = int(counts[:, s].max())
        caps.append(0 if mx == 0 else -(-mx // PAD) * PAD)
    r_pad = int(sum(caps))
    offs = np.concatenate([[0], np.cumsum(caps)]).astype(np.int64)

    in_maps = []
    for c in range(N_CORES):
        xT = np.zeros((D_IN, r_pad), dtype=np.float16)
        for s in range(N_SPECIES):
            n = counts[c, s]
            if n:
                xT[:, offs[s]:offs[s] + n] = values[core_rows[c][s]].T
        in_maps.append({"xT": xT, "w": w_host})

    plan = {"core_rows": core_rows, "counts": counts, "caps": caps,
            "offs": offs, "r_pad": r_pad}
    return in_maps, plan


def _postprocess(results, plan):
    core_rows, counts, offs = plan["core_rows"], plan["counts"], plan["offs"]
    out = np.empty((M_TOTAL, N_OUT), dtype=np.float32)
    for c in range(N_CORES):
        oT = results[c]["outT"].astype(np.float32)
        for s in range(N_SPECIES):
            n = counts[c, s]
            if n:
                out[core_rows[c][s]] = oT[:, offs[s]:offs[s] + n].T
    return out


def kernel(values, species_idx, combining_matrix):
    in_maps, plan = _prepare(values, species_idx, combining_matrix)
    nc = _build_nc(plan["caps"], plan["r_pad"])
    res = run_bass_kernel_spmd(nc, in_maps, list(range(N_CORES)))
    return _postprocess(res.results, plan)


# revision 10
# speedup vs baseline: 1.5031x; 1.2742x over previous
"""Grouped-GEMM (MoE routing) kernel for TRN2, 8 NeuronCores, SPMD.

out[m] = values[m] @ combining_matrix[species_idx[m]]
  values [131072, 128] f32, species_idx [131072] i32, combining_matrix [8, 128, 256] f32

Strategy:
  - Host: counting-sort rows by species; deal each species' rows round-robin
    across the 8 cores so per-core per-species counts are balanced (+-1).
    Each core's rows are packed species-contiguous into a transposed buffer
    xT [128, R_pad] (species segment s zero-padded to a static capacity C[s],
    identical on every core -> one SPMD program).
  - fp16 end-to-end I/O: x, weights and the output cross HBM as fp16 (the
    kernel is DMA-bound; fp32 accumulate in PSUM keeps rel-err ~5e-4).
  - Device (per core): x lives in one persistent SBUF tile [128, R_pad]
    streamed in by a few 4096-col DMAs (8KB/partition lines). All 8 weight
    matrices resident in SBUF ([128, 8*256] fp16 = 4KB/partition). Per
    species s / output half h, the segment is split into near-equal
    supertiles <=2048 cols; each runs 512-col matmuls into a 4-bank PSUM
    tile (bufs=2: uniform supertile sizes keep the two slots strictly
    alternating so PE and the cast engines overlap). PSUM fp32 -> fp16
    casts alternate between DVE and ACT, writing into 4096-col staging
    tiles; a completed staging tile is DMAed (8KB/partition lines) to
    outT [256, R_pad] from the otherwise-idle sync queue.
  - Host: scatter outT columns back to the full [131072, 256] fp32 output.

This does 1x the FLOPs of the reference's 8x masked-matmul formulation and is
DMA-roofline-bound (~13.3 MB/core HBM traffic at fp16).
"""

import numpy as np
from contextlib import ExitStack

import concourse.bass as bass
import concourse.mybir as mybir
import concourse.tile as tile
from concourse import bacc
from concourse.bass_utils import run_bass_kernel_spmd

M_TOTAL = 131072
D_IN = 128
N_OUT = 256
N_SPECIES = 8
N_CORES = 8
PAD = 64           # species segment capacity granularity (rows)
CHUNK = 512        # matmul moving-dim chunk (PSUM bank = 512 fp32 cols)
ST = 1024          # max supertile cols (2 PSUM banks, 4-deep rotation)
IN_PIECE = 4096    # input DMA piece (cols); 8KB/partition lines at fp16
OPIECE = 4096      # output staging piece (cols); 8KB/partition lines
F32 = mybir.dt.float32
F16 = mybir.dt.float16


def _supertiles(cs):
    """Split cs cols into supertiles <= ST (returns sizes)."""
    out = [ST] * (cs // ST)
    if cs % ST:
        out.append(cs % ST)
    return out


def _build_nc(caps, r_pad):
    """Build the SPMD program for one core. caps[s] = padded column count of
    species segment s (same on all cores); r_pad = sum(caps)."""
    nc = bacc.Bacc("TRN2", target_bir_lowering=False, debug=False,
                   num_devices=N_CORES)
    xT = nc.dram_tensor("xT", [D_IN, r_pad], F16, kind="ExternalInput").ap()
    w = nc.dram_tensor("w", [D_IN, N_SPECIES * N_OUT], F16,
                       kind="ExternalInput").ap()
    outT = nc.dram_tensor("outT", [N_OUT, r_pad], F16, kind="ExternalOutput").ap()

    # (species, xT column offset, columns) per species segment
    sched = []
    off = 0
    for s in range(N_SPECIES):
        if caps[s]:
            sched.append((s, off, caps[s]))
            off += caps[s]

    # output staging pieces per h: first small so the output stream starts
    # early, then 8KB-line pieces
    opieces = [min(ST, r_pad)]
    while sum(opieces) < r_pad:
        opieces.append(min(OPIECE, r_pad - sum(opieces)))
    obounds = np.concatenate([[0], np.cumsum(opieces)]).astype(int)

    with tile.TileContext(nc) as tc, ExitStack() as ctx:
        wpool = ctx.enter_context(tc.tile_pool(name="w", bufs=1))
        xpool = ctx.enter_context(tc.tile_pool(name="x", bufs=1))
        opool = ctx.enter_context(tc.tile_pool(name="o", bufs=6))
        pspool = ctx.enter_context(tc.tile_pool(name="ps", bufs=4, space="PSUM"))

        wt = wpool.tile([D_IN, N_SPECIES * N_OUT], F16)
        xt = xpool.tile([D_IN, r_pad], F16)

        # weights first on the sync ring, first x piece on the (still idle)
        # scalar ring so both HWDGE rings ramp in parallel at kernel start
        nc.sync.dma_start(wt, w)
        pieces = [(0, min(ST, r_pad))]
        p0 = pieces[0][1]
        while p0 < r_pad:
            pn = min(IN_PIECE, r_pad - p0)
            pieces.append((p0, pn))
            p0 += pn
        for i, (p0, pn) in enumerate(pieces):
            ieng = nc.scalar if i == 0 else nc.sync
            ieng.dma_start(xt[:, p0:p0 + pn], xT[:, p0:p0 + pn])

        # per h: lazily-created staging tiles and #cols still to be cast
        otile = {}                      # (h, piece idx) -> SBUF tile
        left = {0: list(opieces), 1: list(opieces)}
        n_cast = 0

        def emit_cast(eng, h, g0, ps, ps0, tn):
            """Cast ps[:, ps0:ps0+tn] (fp32) into the h staging tiles at
            global col g0, splitting at piece boundaries; DMA filled pieces."""
            while tn:
                k = int(np.searchsorted(obounds, g0, side="right")) - 1
                c0 = g0 - obounds[k]
                n = min(tn, int(obounds[k + 1]) - g0)
                if (h, k) not in otile:
                    otile[(h, k)] = opool.tile([128, OPIECE], F16, tag="o",
                                               name=f"ot_{h}_{k}")
                ot = otile[(h, k)]
                if eng == 0:
                    nc.vector.tensor_copy(ot[:, c0:c0 + n], ps[:, ps0:ps0 + n])
                else:
                    nc.scalar.copy(ot[:, c0:c0 + n], ps[:, ps0:ps0 + n])
                left[h][k] -= n
                if left[h][k] == 0:
                    pw = int(opieces[k])
                    nc.sync.dma_start(
                        outT[h * 128:(h + 1) * 128,
                             int(obounds[k]):int(obounds[k]) + pw],
                        ot[:, :pw])
                g0 += n
                ps0 += n
                tn -= n

        for (s, off, cs) in sched:
            for h in range(2):
                lhsT = wt[:, s * N_OUT + h * 128: s * N_OUT + h * 128 + 128]
                t0 = 0
                for tn in _supertiles(cs):
                    ps = pspool.tile([128, ST], F32, tag="ps")
                    for j0 in range(0, tn, CHUNK):
                        cj = min(CHUNK, tn - j0)
                        nc.tensor.matmul(ps[:, j0:j0 + cj], lhsT,
                                         xt[:, off + t0 + j0:off + t0 + j0 + cj],
                                         start=True, stop=True)
                    emit_cast(n_cast % 2, h, off + t0, ps, 0, tn)
                    n_cast += 1
                    t0 += tn

    nc.compile()
    return nc


def _prepare(values, species_idx, combining_matrix):
    """Host routing + packing. Returns (in_maps, plan)."""
    values = np.ascontiguousarray(values, dtype=np.float32)
    species_idx = np.asarray(species_idx, dtype=np.int32)
    w_host = np.ascontiguousarray(
        np.asarray(combining_matrix, dtype=np.float32).transpose(1, 0, 2).reshape(
            D_IN, N_SPECIES * N_OUT)
    ).astype(np.float16)

    # per species, deal rows round-robin across cores (balanced +-1)
    core_rows = [[] for _ in range(N_CORES)]   # per core: list of row-index arrays
    counts = np.zeros((N_CORES, N_SPECIES), dtype=np.int64)
    for s in range(N_SPECIES):
        idx = np.nonzero(species_idx == s)[0]
        for c in range(N_CORES):
            sub = idx[c::N_CORES]
            core_rows[c].append(sub)
            counts[c, s] = sub.size

    caps = []
    for s in range(N_SPECIES):
        mx = int(counts[:, s].max())
        caps.append(0 if mx == 0 else -(-mx // PAD) * PAD)
    r_pad = int(sum(caps))
    offs = np.concatenate([[0], np.cumsum(caps)]).astype(np.int64)

    in_maps = []
    for c in range(N_CORES):
        xT = np.zeros((D_IN, r_pad), dtype=np.float16)
        for s in range(N_SPECIES):
            n = counts[c, s]
            if n:
                xT[:, offs[s]:offs[s] + n] = values[core_rows[c][s]].T
        in_maps.append({"xT": xT, "w": w_host})

    plan = {"core_rows": core_rows, "counts": counts, "caps": caps,
            "offs": offs, "r_pad": r_pad}
    return in_maps, plan


def _postprocess(results, plan):
    core_rows, counts, offs = plan["core_rows"], plan["counts"], plan["offs"]
    out = np.empty((M_TOTAL, N_OUT), dtype=np.float32)
    for c in range(N_CORES):
        oT = results[c]["outT"].astype(np.float32)
        for s in range(N_SPECIES):
            n = counts[c, s]
            if n:
                out[core_rows[c][s]] = oT[:, offs[s]:offs[s] + n].T
    return out


def kernel(values, species_idx, combining_matrix):
    in_maps, plan = _prepare(values, species_idx, combining_matrix)
    nc = _build_nc(plan["caps"], plan["r_pad"])
    res = run_bass_kernel_spmd(nc, in_maps, list(range(N_CORES)))
    return _postprocess(res.results, plan)
